# revision 1
# baseline (speedup 1.0000x reference)
"""Multi-head attention (B=2, S=4096, D=512, H=8) on 8 TRN2 NeuronCores.

Sharding: data-parallel over (batch, query-chunk). Core i handles batch
i//4 and query rows (i%4)*1024 .. +1024 of that batch. Each core
computes Q projection for its query chunk, K/V projections for the full
batch (redundantly, 4 cores per batch), full attention for all 8 heads
over its queries, and the output projection for its rows. Output slices
are disjoint -> no collectives; host just concatenates.

Per-core device pipeline (transposed "d-major" layout, bf16 matmuls):
  1. Transposing DMAs (bf16 xbar mode) load x^T [i, t] directly; DMA
     queue ordered to minimize XBAR copy<->transpose mode flips.
  2. Q^T/K^T = W^T.T @ x^T; V = x^T.T @ Wv^T (natural layout), stored
     bf16 with a ones-column per head (V_aug). V and the K projection
     of later head pairs are interleaved into the attention loops.
  3. Per (head-pair, q-tile 512, k-chunk 128): scores^T [k,q] via 2
     row-packed matmuls (contraction d=64, heads at array rows 0-63 /
     64-127, concurrent in the PE array), one ACT exp [128,1024]
     psum->sbuf (scale=1/8), 2 attn@V matmuls lhsT=[V_h|1] [128,65] ->
     psum [65,512]; row 64 accumulates the softmax denominator.
     scores/exp for k+1 are emitted before attn@V of k (software
     pipeline) so the PE never waits on the exp.
  4. Normalize: copy po->sbuf (frees psum banks fast), reciprocal of
     row 64, then (deferred into the next block via pending_slow) a
     rank-1 broadcast matmul (f32r) and scalar_tensor_tensor multiply.
  5. Output projection: out[t,o] = sum_h aot_h^T.T @ Wo_h^T + bias.

Engines in steady state: ACT is saturated by the exp (the softmax
exponentials are the single largest floor: S*S*H*B/8 cores/128 lanes
/1.2GHz = 218us); PE array runs scores+attn@V at a similar rate.
"""

import numpy as np
import ml_dtypes

import concourse.bass as bass
import concourse.tile as tile
from concourse import bacc, mybir
from concourse.bass_utils import run_bass_kernel_spmd

F32 = mybir.dt.float32
F32R = mybir.dt.float32r
BF16 = mybir.dt.bfloat16
MUL = mybir.AluOpType.mult

B, S, D, H = 2, 4096, 512, 8
HD = D // H  # 64
NCORES = 8
QCH = B * S // NCORES  # 1024 query rows per core
TKV = S  # 4096 kv rows per core
IC = D // 128  # 4 contraction chunks
OC = D // 128  # 4 output chunks
QT = 512  # q tile (psum bank limit in fp32)
NQT = QCH // QT  # 2
KCH = TKV // 128  # 32 k chunks


def _build_program():
    nc = bacc.Bacc(
        "TRN2",
        target_bir_lowering=False,
        debug=False,
        enable_asserts=False,
        num_devices=NCORES,
    )
    xq = nc.dram_tensor("xq", [QCH, D], BF16, kind="ExternalInput").ap()
    xkv = nc.dram_tensor("xkv", [TKV, D], BF16, kind="ExternalInput").ap()
    wqt = nc.dram_tensor("wqt", [D, D], BF16, kind="ExternalInput").ap()
    wkt = nc.dram_tensor("wkt", [D, D], BF16, kind="ExternalInput").ap()
    wvt = nc.dram_tensor("wvt", [D, D], BF16, kind="ExternalInput").ap()
    wos = nc.dram_tensor("wos", [HD, H, D], BF16, kind="ExternalInput").ap()
    bqs = nc.dram_tensor("bqs", [128, OC], F32, kind="ExternalInput").ap()
    bks = nc.dram_tensor("bks", [128, OC], F32, kind="ExternalInput").ap()
    bvb = nc.dram_tensor("bvb", [128, D], F32, kind="ExternalInput").ap()
    bob = nc.dram_tensor("bob", [128, D], F32, kind="ExternalInput").ap()
    out = nc.dram_tensor("out", [QCH, D], F32, kind="ExternalOutput").ap()

    with tile.TileContext(nc) as tc:
        with (
            tc.tile_pool(name="consts", bufs=1) as consts,
            tc.tile_pool(name="persist", bufs=1) as persist,
            tc.tile_pool(name="pt", bufs=6) as pt_pool,
            tc.tile_pool(name="aot", bufs=1) as aot_pool,
            tc.tile_pool(name="osb", bufs=2) as osb_pool,
            tc.tile_pool(name="posb", bufs=4) as posb_pool,
            tc.tile_pool(name="small", bufs=4) as small_pool,
            # PSUM (8 banks): "sc" scores 2x2, "acc" 2x1 (transposes,
            # proj, pb, final), "po" 2x1 attn-out accumulators.
            tc.tile_pool(name="ps_sc", bufs=2, space="PSUM") as sc_pool,
            tc.tile_pool(name="ps_acc", bufs=2, space="PSUM") as acc_pool,
            tc.tile_pool(name="ps_po", bufs=2, space="PSUM") as po_pool,
        ):
            # ---- constants ----
            ones64f = consts.tile([1, HD], F32)
            nc.vector.memset(ones64f, 1.0)
            ones64 = consts.tile([1, HD], F32R)
            nc.vector.tensor_copy(ones64, ones64f)
            ones1b = consts.tile([1, 128], BF16)
            nc.vector.memset(ones1b, 1.0)


            # ---- persistent activations ----
            # x_kv^T split per DMA segment so transposing DMAs never
            # serialize against earlier segments' readers
            xtks = [
                persist.tile([128, IC, 1024], BF16, name=f"xtk{s}")
                for s in range(TKV // 1024)
            ]
            xtq = persist.tile([128, IC, QCH], BF16)  # x_q^T
            kt = persist.tile([128, OC, TKV], BF16)  # K^T [o-in-chunk, c, t]
            qt = persist.tile([128, OC, QCH], BF16)  # Q^T
            # V_aug: [t-in-chunk, t-chunk, head, 64 V cols + ones col]
            v_sb = persist.tile([128, KCH, H, HD + 1], BF16)
            nc.vector.memset(v_sb[:, :, :, HD : HD + 1], 1.0)

            # ---- phase T: transposing DMA loads x^T directly (bf16).
            # Order minimizes XBAR mode flips: xq+s0 transposes, then the
            # weight copies, then the remaining segments, then wo.
            SEG = 1024  # t-columns per transposed DMA segment
            wq_sb = consts.tile([128, IC, D], BF16)
            nc.sync.dma_start(wq_sb, wqt.rearrange("(c p) o -> p c o", p=128))
            wk_sb = consts.tile([128, IC, D], BF16)
            nc.sync.dma_start(wk_sb, wkt.rearrange("(c p) o -> p c o", p=128))
            wv_sb = consts.tile([128, IC, D], BF16)
            nc.sync.dma_start(wv_sb, wvt.rearrange("(c p) o -> p c o", p=128))
            bq_sb = consts.tile([128, OC], F32)
            nc.sync.dma_start(bq_sb, bqs)
            bk_sb = consts.tile([128, OC], F32)
            nc.sync.dma_start(bk_sb, bks)
            bvb_sb = consts.tile([128, D], F32)
            nc.sync.dma_start(bvb_sb, bvb)
            bv_row = consts.tile([1, D], BF16)
            nc.vector.tensor_copy(bv_row, bvb_sb[0:1, :])
            bob_sb = consts.tile([128, D], F32)
            nc.sync.dma_start(bob_sb, bob)
            for c in range(IC):
                nc.sync.dma_start_transpose(
                    xtq[:, c, :], xq[:, c * 128 : (c + 1) * 128]
                )
            for c in range(IC):
                nc.sync.dma_start_transpose(
                    xtks[0][:, c, :], xkv[0:SEG, c * 128 : (c + 1) * 128]
                )
            for s in range(1, TKV // SEG):
                for c in range(IC):
                    nc.sync.dma_start_transpose(
                        xtks[s][:, c, :],
                        xkv[s * SEG : (s + 1) * SEG, c * 128 : (c + 1) * 128],
                    )
            wo_sb = consts.tile([HD, H, D], BF16)
            nc.sync.dma_start(wo_sb, wos)

            def v_unit(j):
                # V rows for t-chunk j, all heads: [128 t, 512 d] + bias
                ps = acc_pool.tile([128, D], F32, tag="acc", name=f"v{j}")
                s, jj = divmod(j, 8)
                for i in range(IC):
                    nc.tensor.matmul(
                        ps,
                        xtks[s][:, i, jj * 128 : (jj + 1) * 128],
                        wv_sb[:, i, :],
                        start=(i == 0),
                        stop=(i == IC - 1),
                    )
                nc.vector.tensor_add(
                    v_sb[:, j, :, 0:HD],
                    ps.rearrange("p (h d) -> p h d", h=H),
                    bvb_sb.rearrange("p (h d) -> p h d", h=H),
                )

            def q_unit(c, tt):
                ps = acc_pool.tile([128, 512], F32, tag="acc", name=f"q{c}{tt}")
                for i in range(IC):
                    nc.tensor.matmul(
                        ps,
                        wq_sb[:, i, c * 128 : (c + 1) * 128],
                        xtq[:, i, tt * 512 : (tt + 1) * 512],
                        start=(i == 0),
                        stop=(i == IC - 1),
                    )
                nc.vector.tensor_scalar_add(
                    qt[:, c, tt * 512 : (tt + 1) * 512], ps, bq_sb[:, c : c + 1]
                )

            def k_unit(c, tt):
                ps = acc_pool.tile([128, 512], F32, tag="acc", name=f"k{c}_{tt}")
                s, ss = divmod(tt, 2)
                for i in range(IC):
                    nc.tensor.matmul(
                        ps,
                        wk_sb[:, i, c * 128 : (c + 1) * 128],
                        xtks[s][:, i, ss * 512 : (ss + 1) * 512],
                        start=(i == 0),
                        stop=(i == IC - 1),
                    )
                nc.vector.tensor_scalar_add(
                    kt[:, c, tt * 512 : (tt + 1) * 512], ps, bk_sb[:, c : c + 1]
                )

            def proj_units(c):
                for tt in range(TKV // 512):
                    yield lambda tt=tt: k_unit(c, tt)

            # projection units in x^T-segment dependency order; only Q
            # chunk 0 is needed before pair-0 attention starts
            for tt in range(QCH // 512):
                q_unit(0, tt)
            deferred_q = [
                (lambda c=c, tt=tt: q_unit(c, tt))
                for c in range(1, OC)
                for tt in range(QCH // 512)
            ]
            for tt in range(TKV // 512):
                k_unit(0, tt)
            for j in range(16):
                v_unit(j)

            aots = [aot_pool.tile([HD, H, QT], BF16, name=f"aot{qi}") for qi in (0, 1)]

            # ---- output projection for one 128-row block of q-tile qi ----
            def fin_unit(qi, t4):
                ps = acc_pool.tile([128, D], F32, tag="acc", name=f"f{qi}_{t4}")
                for h in range(H):
                    nc.tensor.matmul(
                        ps,
                        aots[qi][:, h, t4 * 128 : (t4 + 1) * 128],
                        wo_sb[:, h, :],
                        start=(h == 0),
                        stop=(h == H - 1),
                    )
                osb = osb_pool.tile([128, D], F32, tag="osb")
                nc.vector.tensor_add(osb, ps, bob_sb)
                t0 = qi * QT + t4 * 128
                nc.sync.dma_start(out[t0 : t0 + 128, :], osb)

            # ---- attention ----
            # Deferred work (next pair's K proj, normalize tails, output
            # projection) is drained one unit every other k-iteration so
            # the PE never stalls in-order on a slow dependency chain.
            pending = []
            pending_slow = []

            def norm2_unit(c, qi, hh, posb, recip):
                pb = acc_pool.tile(
                    [HD, QT], F32, tag="acc", name=f"pb{c}_{qi}_{hh}"
                )
                nc.tensor.matmul(pb, ones64, recip, start=True, stop=True)
                nc.vector.scalar_tensor_tensor(
                    aots[qi][:, 2 * c + hh, :],
                    pb,
                    1.0,
                    posb[0:HD, :],
                    op0=MUL,
                    op1=MUL,
                )

            for c in range(H // 2):
                if c < H // 2 - 1:
                    pending.extend(proj_units(c + 1))
                if c == 0:
                    pending.extend(deferred_q)
                for qi in range(NQT):
                    qs = qi * QT
                    po = [
                        po_pool.tile([HD + 1, QT], F32, tag="po", name=f"po{c}_{qi}_{hh}")
                        for hh in range(2)
                    ]

                    def scores_exp(k, qs=qs, c=c):
                        pss = sc_pool.tile([128, 2, QT], F32, tag="sc")
                        for hh in range(2):
                            off = hh * HD
                            nc.tensor.matmul(
                                pss[:, hh, :],
                                kt[off : off + HD, c, k * 128 : (k + 1) * 128],
                                qt[off : off + HD, c, qs : qs + QT],
                                start=True,
                                stop=True,
                            )
                        ptile = pt_pool.tile([128, 2, QT], BF16, tag="pt")
                        nc.scalar.activation(
                            ptile, pss, mybir.ActivationFunctionType.Exp,
                            scale=1.0 / np.sqrt(HD),
                        )
                        return ptile

                    # software pipeline: scores/exp for k+1 are emitted
                    # before attn@V of k so PE never stalls on the exp
                    ptile = scores_exp(0)
                    for k in range(KCH):
                        nxt = scores_exp(k + 1) if k + 1 < KCH else None
                        for hh in range(2):
                            nc.tensor.matmul(
                                po[hh],
                                v_sb[:, k, 2 * c + hh, :],
                                ptile[:, hh, :],
                                start=(k == 0),
                                stop=(k == KCH - 1),
                            )
                        ptile = nxt
                        if c == 0 and qi == 0 and k < 16:
                            v_unit(k + 16)
                        elif k >= 15 and k % 3 == 0 and pending_slow:
                            pending_slow.pop(0)()
                        elif k >= 3 and k % 2 == 1 and pending:
                            pending.pop(0)()
                    # free the po banks quickly; defer the slow
                    # recip/broadcast/multiply chain into the next block
                    posbs = []
                    for hh in range(2):
                        posb = posb_pool.tile(
                            [HD + 1, QT], F32, tag="posb", name=f"posb{c}_{qi}_{hh}"
                        )
                        nc.vector.tensor_copy(posb, po[hh])
                        posbs.append(posb)
                    for hh in range(2):
                        recip = small_pool.tile([1, QT], F32R, tag="recip")
                        with nc.allow_low_precision(reason="f32r recip"):
                            nc.vector.reciprocal(recip, posbs[hh][HD : HD + 1, :])
                        pending_slow.append(
                            lambda c=c, qi=qi, hh=hh, posb=posbs[hh], recip=recip: (
                                norm2_unit(c, qi, hh, posb, recip)
                            )
                        )
                    if c == H // 2 - 1 and qi == 0:
                        pending_slow.extend(
                            lambda t4=t4: fin_unit(0, t4) for t4 in range(QT // 128)
                        )
            for u in pending + pending_slow:
                u()
            for t4 in range(QT // 128):
                fin_unit(1, t4)

    nc.compile()
    return nc


_NC_CACHE = None


def _get_program():
    global _NC_CACHE
    if _NC_CACHE is None:
        _NC_CACHE = _build_program()
    return _NC_CACHE


def prepare_in_maps(x, Wq, bq, Wk, bk, Wv, bv, Wo, bo):
    bf = ml_dtypes.bfloat16
    x = np.ascontiguousarray(np.asarray(x, dtype=np.float32)).astype(bf)
    sh = {
        "wqt": np.ascontiguousarray(np.asarray(Wq, np.float32).T).astype(bf),
        "wkt": np.ascontiguousarray(np.asarray(Wk, np.float32).T).astype(bf),
        "wvt": np.ascontiguousarray(np.asarray(Wv, np.float32).T).astype(bf),
        "wos": np.ascontiguousarray(
            np.asarray(Wo, np.float32).T.reshape(H, HD, D).transpose(1, 0, 2)
        ).astype(bf),
        "bqs": np.ascontiguousarray(np.asarray(bq, np.float32).reshape(OC, 128).T),
        "bks": np.ascontiguousarray(np.asarray(bk, np.float32).reshape(OC, 128).T),
        "bvb": np.ascontiguousarray(
            np.broadcast_to(np.asarray(bv, np.float32), (128, D))
        ),
        "bob": np.ascontiguousarray(
            np.broadcast_to(np.asarray(bo, np.float32), (128, D))
        ),
    }
    in_maps = []
    for core in range(NCORES):
        b = core // (NCORES // B)
        qs = (core % (NCORES // B)) * QCH
        m = dict(sh)
        m["xq"] = np.ascontiguousarray(x[b, qs : qs + QCH, :])
        m["xkv"] = np.ascontiguousarray(x[b])
        in_maps.append(m)
    return in_maps


def assemble(results):
    out = np.empty((B, S, D), dtype=np.float32)
    for core in range(NCORES):
        b = core // (NCORES // B)
        qs = (core % (NCORES // B)) * QCH
        out[b, qs : qs + QCH, :] = results[core]["out"]
    return out


def kernel(x, Wq, bq, Wk, bk, Wv, bv, Wo, bo):
    in_maps = prepare_in_maps(x, Wq, bq, Wk, bk, Wv, bv, Wo, bo)
    nc = _get_program()
    res = run_bass_kernel_spmd(nc, in_maps, core_ids=list(range(NCORES)))
    return assemble(res.results)



# revision 7
# speedup vs baseline: 1.2502x; 1.2502x over previous
"""Multi-head attention (B=2, S=4096, D=512, H=8) on 8 TRN2 NeuronCores.

Sharding: (batch, head-pair) tensor parallel. Core i handles batch i//4
and heads 2*(i%4), 2*(i%4)+1. Each core computes Q/K/V projections only
for its two heads (1/4 of the projection work, no redundancy), full
S x S attention for those heads over all 4096 queries, and a PARTIAL
output projection out_partial = aot_pair^T @ Wo_pair + bo/4. The host
sums the 4 partials per batch (f32) -- no device collectives.

Per-core device pipeline (d-major transposed layout, bf16 matmuls):
  1. Transposing DMAs load x^T [d, t] in 4 segments of 1024 t.
  2. Q^T/K^T = Wpair x^T (f=512 matmuls, 128-row output = both heads);
     V natural [t, pair-dv] with a ones-column per head (V_aug).
  3. Per (q-tile 512, k-chunk 128): 2 row-packed score matmuls
     (c=64, heads at PE rows 0-63/64-127 run concurrently), one ACT
     exp [128,1024] psum->sbuf (scale=1/8), 2 attn@V matmuls
     lhsT=[V_h|1] [128,65] -> po [65,512]; row 64 accumulates the
     softmax denominator. scores/exp for k+1 are emitted before attn@V
     of k (software pipeline) so ACT -- the bottleneck engine -- is
     never starved.
  4. Normalize per q-tile: copy po->sbuf (frees psum), fast-approx
     reciprocal of row 64, rank-1 broadcast matmul (f32r) and
     scalar_tensor_tensor multiply -> aot [128(2 heads' d), 512].
  5. Partial output projection: one c=128 matmul per 128-row t-chunk
     (both heads contracted at once), + bo/4, DMA out f32.

Steady state is ACT(exp)-bound: 256 instrs x ~1.0us ~= 260us; PE has
~80us of slack which absorbs projections and normalization.
"""

import numpy as np
import ml_dtypes

import concourse.bass as bass
import concourse.tile as tile
from concourse import bacc, mybir
from concourse.bass_utils import run_bass_kernel_spmd

F32 = mybir.dt.float32
F32R = mybir.dt.float32r
FP16 = mybir.dt.float16
BF16 = mybir.dt.bfloat16
MUL = mybir.AluOpType.mult

B, S, D, H = 2, 4096, 512, 8
HD = D // H  # 64
NCORES = 8
PAIRS = 4  # head-pairs; one per core (per batch)
IC = D // 128  # 4 contraction chunks over d_model
QT = 512  # q tile
NQT = S // QT  # 8
KCH = S // 128  # 32 k chunks
SEG = 1024  # t-columns per transposed DMA segment
NSEG = S // SEG  # 4


def _build_program():
    nc = bacc.Bacc(
        "TRN2",
        target_bir_lowering=False,
        debug=False,
        enable_asserts=False,
        num_devices=NCORES,
    )
    x = nc.dram_tensor("x", [S, D], BF16, kind="ExternalInput").ap()
    wqt = nc.dram_tensor("wqt", [D, 128], BF16, kind="ExternalInput").ap()
    wkt = nc.dram_tensor("wkt", [D, 128], BF16, kind="ExternalInput").ap()
    wvt = nc.dram_tensor("wvt", [D, 128], BF16, kind="ExternalInput").ap()
    wos = nc.dram_tensor("wos", [128, D], BF16, kind="ExternalInput").ap()
    bqs = nc.dram_tensor("bqs", [128, 1], F32, kind="ExternalInput").ap()
    bks = nc.dram_tensor("bks", [128, 1], F32, kind="ExternalInput").ap()
    bvb = nc.dram_tensor("bvb", [128, 128], F32, kind="ExternalInput").ap()
    bob = nc.dram_tensor("bob", [128, D], F32, kind="ExternalInput").ap()
    out = nc.dram_tensor("out", [S, D], F32, kind="ExternalOutput").ap()

    with tile.TileContext(nc) as tc:
        with (
            tc.tile_pool(name="consts", bufs=1) as consts,
            tc.tile_pool(name="persist", bufs=1) as persist,
            tc.tile_pool(name="pt", bufs=6) as pt_pool,
            tc.tile_pool(name="aot", bufs=2) as aot_pool,
            tc.tile_pool(name="osb", bufs=4) as osb_pool,
            tc.tile_pool(name="posb", bufs=4) as posb_pool,
            tc.tile_pool(name="small", bufs=4) as small_pool,
            # PSUM (8 banks): sc 2x2, po 2x1, acc 2x1
            tc.tile_pool(name="ps_sc", bufs=2, space="PSUM") as sc_pool,
            tc.tile_pool(name="ps_po", bufs=2, space="PSUM") as po_pool,
            tc.tile_pool(name="ps_acc", bufs=2, space="PSUM") as acc_pool,
        ):
            # ---- constants ----
            ones64f = consts.tile([1, HD], F32)
            nc.vector.memset(ones64f, 1.0)
            ones64 = consts.tile([1, HD], FP16)
            nc.vector.tensor_copy(ones64, ones64f)

            # ---- persistent activations ----
            xtks = [
                persist.tile([128, IC, SEG], BF16, name=f"xtk{s}")
                for s in range(NSEG)
            ]
            kt = persist.tile([128, S], BF16)  # K^T pair [dv, t]
            qt = persist.tile([128, S], BF16)  # Q^T pair
            # V_aug: [t-in-chunk, t-chunk, head-in-pair, 64 V cols + ones]
            v_sb = persist.tile([128, KCH, 2, HD + 1], BF16)
            nc.vector.memset(v_sb[:, :, :, HD : HD + 1], 1.0)

            # ---- DMAs: x^T seg 0 first, then weights, then segs 1-3 ----
            for c in range(IC):
                nc.sync.dma_start_transpose(
                    xtks[0][:, c, :], x[0:SEG, c * 128 : (c + 1) * 128]
                )
            wq_sb = consts.tile([128, IC, 128], BF16)
            nc.sync.dma_start(wq_sb, wqt.rearrange("(c p) o -> p c o", p=128))
            wk_sb = consts.tile([128, IC, 128], BF16)
            nc.sync.dma_start(wk_sb, wkt.rearrange("(c p) o -> p c o", p=128))
            wv_sb = consts.tile([128, IC, 128], BF16)
            nc.sync.dma_start(wv_sb, wvt.rearrange("(c p) o -> p c o", p=128))
            bq_sb = consts.tile([128, 1], F32)
            nc.sync.dma_start(bq_sb, bqs)
            bk_sb = consts.tile([128, 1], F32)
            nc.sync.dma_start(bk_sb, bks)
            bvb_sb = consts.tile([128, 128], F32)
            nc.sync.dma_start(bvb_sb, bvb)
            bob_sb = consts.tile([128, D], F32)
            nc.sync.dma_start(bob_sb, bob)
            wo_sb = consts.tile([128, D], BF16)
            nc.sync.dma_start(wo_sb, wos)
            for s in range(1, NSEG):
                for c in range(IC):
                    nc.sync.dma_start_transpose(
                        xtks[s][:, c, :],
                        x[s * SEG : (s + 1) * SEG, c * 128 : (c + 1) * 128],
                    )

            # ---- projection units ----
            def q_unit(tt):
                ps = acc_pool.tile([128, QT], F32, tag="acc", name=f"q{tt}")
                s, ss = divmod(tt, 2)
                for i in range(IC):
                    nc.tensor.matmul(
                        ps,
                        wq_sb[:, i, :],
                        xtks[s][:, i, ss * QT : (ss + 1) * QT],
                        start=(i == 0),
                        stop=(i == IC - 1),
                    )
                nc.vector.tensor_scalar_add(
                    qt[:, tt * QT : (tt + 1) * QT], ps, bq_sb[:, 0:1]
                )

            def k_unit(tt):
                ps = acc_pool.tile([128, QT], F32, tag="acc", name=f"k{tt}")
                s, ss = divmod(tt, 2)
                for i in range(IC):
                    nc.tensor.matmul(
                        ps,
                        wk_sb[:, i, :],
                        xtks[s][:, i, ss * QT : (ss + 1) * QT],
                        start=(i == 0),
                        stop=(i == IC - 1),
                    )
                nc.vector.tensor_scalar_add(
                    kt[:, tt * QT : (tt + 1) * QT], ps, bk_sb[:, 0:1]
                )

            def v_unit(j):
                # V rows for t-chunk j, both heads: [128 t, 128 dv] + bias
                ps = acc_pool.tile([128, 128], F32, tag="acc", name=f"v{j}")
                s, jj = divmod(j, 8)
                for i in range(IC):
                    nc.tensor.matmul(
                        ps,
                        xtks[s][:, i, jj * 128 : (jj + 1) * 128],
                        wv_sb[:, i, :],
                        start=(i == 0),
                        stop=(i == IC - 1),
                    )
                nc.vector.tensor_add(
                    v_sb[:, j, :, 0:HD],
                    ps.rearrange("p (h d) -> p h d", h=2),
                    bvb_sb.rearrange("p (h d) -> p h d", h=2),
                )

            # upfront: just enough for attention (qi=0) to start
            q_unit(0)
            k_unit(0)
            v_unit(0)
            v_unit(1)
            k_unit(1)
            v_unit(2)
            v_unit(3)
            pending = []
            for u in (4, 5, 6, 7):
                pending.append(lambda j=u: v_unit(j))
            pending.append(lambda: q_unit(1))
            for tt in range(2, 8):  # k segs with their v chunks
                pending.append(lambda tt=tt: k_unit(tt))
                for j in range(4 * tt, 4 * tt + 4):
                    pending.append(lambda j=j: v_unit(j))
            for tt in range(2, 8):
                pending.append(lambda tt=tt: q_unit(tt))

            pending_slow = []

            aots = {}

            def norm_unit(qi, posbN, recs):
                pb2 = acc_pool.tile([128, QT], F32, tag="acc", name=f"pb{qi}")
                nc.tensor.matmul(
                    pb2[0:HD, :], ones64, recs[0], start=True, stop=True
                )
                nc.tensor.matmul(
                    pb2[HD:128, :], ones64, recs[1], start=True, stop=True
                )
                nc.vector.scalar_tensor_tensor(
                    aots[qi], pb2, 1.0, posbN, op0=MUL, op1=MUL
                )

            def fin_unit(qi, t4):
                ps = acc_pool.tile([128, D], F32, tag="acc", name=f"f{qi}_{t4}")
                nc.tensor.matmul(
                    ps,
                    aots[qi][:, t4 * 128 : (t4 + 1) * 128],
                    wo_sb,
                    start=True,
                    stop=True,
                )
                osb = osb_pool.tile([128, D], F32, tag="osb")
                nc.vector.tensor_add(osb, ps, bob_sb)
                t0 = qi * QT + t4 * 128
                nc.sync.dma_start(out[t0 : t0 + 128, :], osb)

            # ---- attention ----
            for qi in range(NQT):
                qs = qi * QT
                aots[qi] = aot_pool.tile(
                    [128, QT], BF16, tag="aot", name=f"aot{qi}"
                )
                po = [
                    po_pool.tile([HD + 1, QT], F32, tag="po", name=f"po{qi}_{hh}")
                    for hh in range(2)
                ]

                def scores_exp(k, qs=qs):
                    pss = sc_pool.tile([128, 2, QT], F32, tag="sc")
                    for hh in range(2):
                        off = hh * HD
                        nc.tensor.matmul(
                            pss[:, hh, :],
                            kt[off : off + HD, k * 128 : (k + 1) * 128],
                            qt[off : off + HD, qs : qs + QT],
                            start=True,
                            stop=True,
                        )
                    ptile = pt_pool.tile([128, 2, QT], BF16, tag="pt")
                    nc.scalar.activation(
                        ptile, pss, mybir.ActivationFunctionType.Exp,
                        scale=1.0 / np.sqrt(HD),
                    )
                    return ptile

                # software pipeline: scores/exp for k+1 before attn@V of k
                ptile = scores_exp(0)
                for k in range(KCH):
                    it = qi * KCH + k
                    nxt = scores_exp(k + 1) if k + 1 < KCH else None
                    for hh in range(2):
                        nc.tensor.matmul(
                            po[hh],
                            v_sb[:, k, hh, :],
                            ptile[:, hh, :],
                            start=(k == 0),
                            stop=(k == KCH - 1),
                        )
                    ptile = nxt
                    # drain deferred work into PE's slack
                    if it >= 2 and it % 5 != 0 and pending:
                        pending.pop(0)()
                        if it < 24 and pending:
                            pending.pop(0)()
                    elif it % 5 == 0 and pending_slow:
                        pending_slow.pop(0)()

                # free po banks fast; defer the slow normalize + fin chain
                posbN = posb_pool.tile([128, QT], F32, tag="posb", name=f"posb{qi}")
                nc.vector.tensor_copy(posbN[0:HD, :], po[0][0:HD, :])
                nc.vector.tensor_copy(posbN[HD : 2 * HD, :], po[1][0:HD, :])
                recs = []
                for hh in range(2):
                    db = small_pool.tile([1, QT], F32, tag="db")
                    nc.vector.tensor_copy(db, po[hh][HD : HD + 1, :])
                    recf = small_pool.tile([1, QT], F32, tag="recf")
                    nc.vector.reciprocal_approx_fast(recf, db)
                    rec = small_pool.tile([1, QT], FP16, tag="rec")
                    nc.vector.tensor_copy(rec, recf)
                    recs.append(rec)
                pending_slow.append(
                    lambda qi=qi, posbN=posbN, recs=recs: norm_unit(qi, posbN, recs)
                )
                pending_slow.extend(
                    lambda qi=qi, t4=t4: fin_unit(qi, t4) for t4 in range(4)
                )

            for u in pending + pending_slow:
                u()

    nc.compile()
    return nc


_NC_CACHE = None


def _get_program():
    global _NC_CACHE
    if _NC_CACHE is None:
        _NC_CACHE = _build_program()
    return _NC_CACHE


def prepare_in_maps(x, Wq, bq, Wk, bk, Wv, bv, Wo, bo):
    bf = ml_dtypes.bfloat16
    x = np.ascontiguousarray(np.asarray(x, dtype=np.float32)).astype(bf)
    wqT = np.asarray(Wq, np.float32).T  # [D in, D out-rows]
    wkT = np.asarray(Wk, np.float32).T
    wvT = np.asarray(Wv, np.float32).T
    woT = np.asarray(Wo, np.float32).T  # [D dv, D out]
    bq = np.asarray(bq, np.float32)
    bk = np.asarray(bk, np.float32)
    bv = np.asarray(bv, np.float32)
    bo = np.asarray(bo, np.float32)
    in_maps = []
    for core in range(NCORES):
        b = core // PAIRS
        hp = core % PAIRS
        pr = slice(hp * 128, (hp + 1) * 128)
        m = {
            "x": np.ascontiguousarray(x[b]),
            "wqt": np.ascontiguousarray(wqT[:, pr]).astype(bf),
            "wkt": np.ascontiguousarray(wkT[:, pr]).astype(bf),
            "wvt": np.ascontiguousarray(wvT[:, pr]).astype(bf),
            "wos": np.ascontiguousarray(woT[pr, :]).astype(bf),
            "bqs": np.ascontiguousarray(bq[pr].reshape(128, 1)),
            "bks": np.ascontiguousarray(bk[pr].reshape(128, 1)),
            "bvb": np.ascontiguousarray(
                np.broadcast_to(bv[pr][None, :], (128, 128))
            ),
            "bob": np.ascontiguousarray(
                np.broadcast_to(bo[None, :] * 0.25, (128, D))
            ),
        }
        in_maps.append(m)
    return in_maps


def assemble(results):
    out = np.empty((B, S, D), dtype=np.float32)
    for b in range(B):
        acc = results[b * PAIRS]["out"].astype(np.float32, copy=True)
        for hp in range(1, PAIRS):
            acc += results[b * PAIRS + hp]["out"]
        out[b] = acc
    return out


def kernel(x, Wq, bq, Wk, bk, Wv, bv, Wo, bo):
    in_maps = prepare_in_maps(x, Wq, bq, Wk, bk, Wv, bv, Wo, bo)
    nc = _get_program()
    res = run_bass_kernel_spmd(nc, in_maps, core_ids=list(range(NCORES)))
    return assemble(res.results)
